# revision 3
# baseline (speedup 1.0000x reference)
"""MLA attention kernel for 8 Trainium2 NeuronCores.

Sharding: core i -> batch b = i//4, head group hg = i%4 (32 heads each).
Latent down-projections replicated within a batch group; Wq_up/Wq_rope/
Wk_up/Wv_up/Wo sharded by head.  Host sums the 4 partial outputs per batch.

Device program (identical on all cores, SPMD over different data):
  - all matmuls bf16 with fp32 PSUM accumulation
  - projections computed feature-major (features on partitions) so that
    attention scores S^T[k, q] = kT.T @ qT need no transposes
  - softmax: exp on ScalarE (scale 1/sqrt(96) folded in, no max subtraction:
    scores are ~N(0,1)), denominator via an appended ones-column of V in the
    attn@V matmul, division via DVE fast-reciprocal + gpsimd partition
    broadcast
  - engines execute their queues in order, so the per-group instruction
    stream software-pipelines: group g's attention matmuls (which stall on
    ScalarE exps) are interleaved with group g+1's projection matmuls so
    the PE always has ready work; the Wo epilogue reuses the same tile-pool
    tags so its first weight slab DMA prefetches during the last group
"""

import sys

sys.path.insert(0, "/opt/trn_rl_repo")

import numpy as np
import ml_dtypes

import concourse.bass as bass
import concourse.tile as tile
from concourse import bacc, mybir
from concourse.bass_utils import run_bass_kernel_spmd

P = 128
T = 1024          # tokens per batch
DM = 4096         # d_model
KX = DM // P      # 32 feature chunks of x
LAT = 512         # latent dim
LC = LAT // P     # 4 latent chunks
NHC = 32          # heads per core
DH = 32           # head dim (compressed part)
DR = 64           # rope dim per head
NB = 2            # batch
SCALE = 1.0 / float(np.sqrt(DH + DR))

BF = mybir.dt.bfloat16
F32 = mybir.dt.float32

_CACHE = {}


def _build_program():
    nc = bacc.Bacc("TRN2", target_bir_lowering=False, num_devices=8)

    xT = nc.declare_dram_parameter("xT", [DM, T], BF, isOutput=False)
    wqd = nc.declare_dram_parameter("wqd", [DM, LAT], BF, isOutput=False)
    wkvd = nc.declare_dram_parameter("wkvd", [DM, LAT], BF, isOutput=False)
    wqu = nc.declare_dram_parameter("wqu", [LAT, NHC * DH], BF, isOutput=False)
    wku = nc.declare_dram_parameter("wku", [LAT, NHC * DH], BF, isOutput=False)
    wvu = nc.declare_dram_parameter("wvu", [LAT, NHC * DH], BF, isOutput=False)
    wqr = nc.declare_dram_parameter("wqr", [DM, NHC * DR], BF, isOutput=False)
    wkr = nc.declare_dram_parameter("wkr", [DM, DR], BF, isOutput=False)
    wo = nc.declare_dram_parameter("wo", [NHC * DH, DM], BF, isOutput=False)
    out = nc.declare_dram_parameter("out", [T, DM], F32, isOutput=True)

    from contextlib import ExitStack

    with tile.TileContext(nc) as tc, ExitStack() as octx:
        const = octx.enter_context(tc.tile_pool(name="const", bufs=1))

        # Resident tensors (bf16): x transposed, Wk_rope, projections outputs.
        xT_sb = const.tile([P, KX, T], BF, name="xT_sb")
        xT_r = xT[:].rearrange("(ko p) t -> p ko t", p=P)
        wkr_sb = const.tile([P, KX, DR], BF, name="wkr_sb")
        wvu_sb = const.tile([P, LC, NHC * DH], BF, name="wvu_sb")

        cq_sb = const.tile([P, LC, T], BF, name="cq_sb")      # c_q^T
        ckv_sb = const.tile([P, LC, T], BF, name="ckv_sb")    # c_kv^T
        kr_sb = const.tile([DR, T], BF, name="kr_sb")         # k_rope^T (shared)
        # v token-major, per (key-chunk, head): cols 0:32 = v, col 32 = ones
        v_sb = const.tile([P, 8, NHC, 34], BF, name="v_sb")
        # attention output, feature-major: head h -> [32*(h%4):.., h//4, :]
        aout_sb = const.tile([P, 8, T], BF, name="aout_sb")

        nc.vector.memset(v_sb[:, :, :, 32:33], 1.0)

        with ExitStack() as ctx:
            wpool = ctx.enter_context(tc.tile_pool(name="wpool", bufs=2))
            cpp = ctx.enter_context(tc.tile_pool(name="cpp", bufs=3, space="PSUM"))
            qkpool = ctx.enter_context(tc.tile_pool(name="qkpool", bufs=16))
            ppool = ctx.enter_context(tc.tile_pool(name="ppool", bufs=2))
            spp = ctx.enter_context(tc.tile_pool(name="spp", bufs=3, space="PSUM"))
            avp = ctx.enter_context(tc.tile_pool(name="avp", bufs=2, space="PSUM"))
            rpool = ctx.enter_context(tc.tile_pool(name="rpool", bufs=2))

            # x loaded first (PE starts only once data is in — starting it
            # earlier just runs sparse work at the cold HAM half-clock).
            for kq in range(4):
                nc.sync.dma_start(
                    out=xT_sb[:, kq * 8:(kq + 1) * 8, :],
                    in_=xT_r[:, kq * 8:(kq + 1) * 8, :],
                )
            nc.sync.dma_start(
                out=wkr_sb[:], in_=wkr[:].rearrange("(ko p) d -> p ko d", p=P)
            )
            nc.sync.dma_start(
                out=wvu_sb[:], in_=wvu[:].rearrange("(c p) m -> p c m", p=P)
            )

            # ---- Phase B: latent down-projections (feature-major outputs) ----
            for wd, cdst in ((wqd, cq_sb), (wkvd, ckv_sb)):
                for m in range(LC):
                    wslab = wpool.tile([P, KX, P], BF, tag="wqrs", name="bslab")
                    b_src = wd[:, m * P:(m + 1) * P].rearrange(
                        "(ko p) m -> p ko m", p=P
                    )
                    nc.sync.dma_start(out=wslab[:, 0:16, :], in_=b_src[:, 0:16, :])
                    nc.sync.dma_start(out=wslab[:, 16:KX, :], in_=b_src[:, 16:KX, :])
                    for hf in range(2):
                        ps = cpp.tile([P, 512], F32, tag="cps")
                        for k in range(KX):
                            nc.tensor.matmul(
                                ps[:],
                                wslab[:, k, :],
                                xT_sb[:, k, hf * 512:(hf + 1) * 512],
                                start=(k == 0),
                                stop=(k == KX - 1),
                            )
                        nc.vector.tensor_copy(
                            out=cdst[:, m, hf * 512:(hf + 1) * 512], in_=ps[:]
                        )

            # k_rope^T [64, T]
            for hf in range(2):
                ps = cpp.tile([P, 512], F32, tag="cps")
                for k in range(KX):
                    nc.tensor.matmul(
                        ps[:DR, :],
                        wkr_sb[:, k, :],
                        xT_sb[:, k, hf * 512:(hf + 1) * 512],
                        start=(k == 0),
                        stop=(k == KX - 1),
                    )
                nc.vector.tensor_copy(
                    out=kr_sb[:, hf * 512:(hf + 1) * 512], in_=ps[:DR, :]
                )

            # ---- Phase V: v = c_kv @ Wv_up (token-major), interleaved heads ----
            for tt in range(8):
                for hf in range(2):
                    ps = cpp.tile([P, 512], F32, tag="cps")
                    for lc in range(LC):
                        nc.tensor.matmul(
                            ps[:],
                            ckv_sb[:, lc, tt * P:(tt + 1) * P],
                            wvu_sb[:, lc, hf * 512:(hf + 1) * 512],
                            start=(lc == 0),
                            stop=(lc == LC - 1),
                        )
                    # scatter 16 heads x 32 dims into v_sb[:, tt, h, 0:32]
                    nc.vector.tensor_copy(
                        out=v_sb[:, tt, hf * 16:(hf + 1) * 16, 0:32],
                        in_=ps[:].rearrange("p (h d) -> p h d", h=16),
                    )

            # ---- Phase C+D: project q/k per group of 4 heads, then attend.
            # proj(g) is materialized as a list of closures, each emitting a
            # small burst of PE work; attn(g-1) paces through the list so the
            # PE stream always has independent matmuls between exp-gated
            # score/av matmuls.
            def make_proj(g):
                qt = []
                kt = []
                for j in range(4):
                    qtj = qkpool.tile([P, T], BF, tag="qkt", name=f"qt{g}_{j}")
                    ktj = qkpool.tile([P, T], BF, tag="qkt", name=f"kt{g}_{j}")
                    qt.append(qtj)
                    kt.append(ktj)

                chunks = []

                def kr_copies():
                    for j in range(4):
                        nc.vector.tensor_copy(out=kt[j][0:DR, :], in_=kr_sb[:])
                chunks.append(kr_copies)

                # q_rope: wqr m-slabs 2g, 2g+1 -> heads (2*ms, 2*ms+1) rows 0:64
                for s in range(2):
                    wslab = wpool.tile([P, KX, P], BF, tag="wqrs", name="qrslab")
                    qr_src = wqr[:, (2 * g + s) * P:(2 * g + s + 1) * P].rearrange(
                        "(ko p) m -> p ko m", p=P
                    )

                    def qr_dma(wslab=wslab, qr_src=qr_src):
                        nc.sync.dma_start(out=wslab[:, 0:16, :], in_=qr_src[:, 0:16, :])
                        nc.sync.dma_start(out=wslab[:, 16:KX, :], in_=qr_src[:, 16:KX, :])
                    chunks.append(qr_dma)

                    for hf in range(2):
                        ps = cpp.tile([P, 512], F32, tag="cps", name="qr_ps")
                        for k0 in range(0, KX, 8):
                            def qr_mms(ps=ps, wslab=wslab, hf=hf, k0=k0):
                                for k in range(k0, k0 + 8):
                                    nc.tensor.matmul(
                                        ps[:],
                                        wslab[:, k, :],
                                        xT_sb[:, k, hf * 512:(hf + 1) * 512],
                                        start=(k == 0),
                                        stop=(k == KX - 1),
                                    )
                            chunks.append(qr_mms)

                        def qr_copy(ps=ps, hf=hf, s=s):
                            sl = slice(hf * 512, (hf + 1) * 512)
                            nc.vector.tensor_copy(out=qt[2 * s][0:DR, sl], in_=ps[0:DR, :])
                            nc.vector.tensor_copy(out=qt[2 * s + 1][0:DR, sl], in_=ps[DR:P, :])
                        chunks.append(qr_copy)

                # q_c / k_c: up-projection slab g (128 cols = 4 heads) rows 64:96
                for wu, dst, src in ((wqu, qt, cq_sb), (wku, kt, ckv_sb)):
                    wslab = wpool.tile([P, LC, P], BF, tag="wups", bufs=3, name="upslab")

                    def up_dma(wslab=wslab, wu=wu):
                        nc.sync.dma_start(
                            out=wslab[:],
                            in_=wu[:, g * P:(g + 1) * P].rearrange(
                                "(c p) m -> p c m", p=P
                            ),
                        )
                    chunks.append(up_dma)

                    for hf in range(2):
                        def up_mms_copy(wslab=wslab, dst=dst, src=src, hf=hf):
                            ps = cpp.tile([P, 512], F32, tag="cps", name="up_ps")
                            for lc in range(LC):
                                nc.tensor.matmul(
                                    ps[:],
                                    wslab[:, lc, :],
                                    src[:, lc, hf * 512:(hf + 1) * 512],
                                    start=(lc == 0),
                                    stop=(lc == LC - 1),
                                )
                            sl = slice(hf * 512, (hf + 1) * 512)
                            for j in range(4):
                                nc.vector.tensor_copy(
                                    out=dst[j][DR:DR + DH, sl],
                                    in_=ps[j * DH:(j + 1) * DH, :],
                                )
                        chunks.append(up_mms_copy)

                return chunks, qt, kt

            def emit_attn(g, qt, kt, filler):
                # filler: list of closures to pace through this group's attn
                done = [0]
                points = 8 * 17  # weave points: per unit 8 score + 8 av + 1
                pt = [0]

                def weave():
                    pt[0] += 1
                    want = (pt[0] * len(filler)) // points
                    while done[0] < want:
                        filler[done[0]]()
                        done[0] += 1

                for j in range(4):
                    h = 4 * g + j
                    for qc in range(2):
                        qsl = slice(qc * 512, (qc + 1) * 512)
                        probs = ppool.tile([P, 8, 512], BF, tag="probs", name="probs")
                        for kc in range(8):
                            sp = spp.tile([P, 512], F32, tag="sps", name="sps")
                            nc.tensor.matmul(
                                sp[:],
                                kt[j][0:96, kc * P:(kc + 1) * P],
                                qt[j][0:96, qsl],
                                start=True,
                                stop=True,
                            )
                            nc.scalar.activation(
                                out=probs[:, kc, :],
                                in_=sp[:],
                                func=mybir.ActivationFunctionType.Exp,
                                scale=SCALE,
                            )
                            weave()
                        av = avp.tile([33, 512], F32, tag="avp", name="av")
                        for kc in range(8):
                            nc.tensor.matmul(
                                av[:],
                                v_sb[:, kc, h, 0:33],
                                probs[:, kc, :],
                                start=(kc == 0),
                                stop=(kc == 7),
                            )
                            weave()
                        # denominator must be copied out of PSUM before the
                        # bit-trick reciprocal (it needs IEEE fp32 bit layout;
                        # PSUM's e10m23 accumulator bits break BITWISE_NOT)
                        den = rpool.tile([1, 512], F32, tag="dc", name="den")
                        nc.vector.tensor_copy(out=den[:], in_=av[32:33, :])
                        recip = rpool.tile([1, 512], F32, tag="rc", name="recip")
                        nc.vector.reciprocal_approx_fast(out=recip[:], in_=den[:])
                        rrep = rpool.tile([DH, 512], F32, tag="rr", name="rrep")
                        nc.gpsimd.partition_broadcast(rrep[:], recip[:])
                        nc.vector.tensor_mul(
                            out=aout_sb[j * DH:(j + 1) * DH, g, qsl],
                            in0=av[0:DH, :],
                            in1=rrep[:],
                        )
                        weave()
                # drain any leftover filler
                while done[0] < len(filler):
                    filler[done[0]]()
                    done[0] += 1

            chunks, qt, kt = make_proj(0)
            for c in chunks:
                c()
            for g in range(8):
                if g < 7:
                    nchunks, nqt, nkt = make_proj(g + 1)
                else:
                    # last group: prefetch the first Wo slab as filler
                    nchunks = []
                emit_attn(g, qt, kt, nchunks)
                if g < 7:
                    qt, kt = nqt, nkt

            # ---- Phase E: out = aout^T @ Wo (token-major), Wo streamed once.
            # Reuses the wqrs slab slots (8 KiB) and cps psum slots so the
            # first weight DMA prefetches while group 7's attention drains.
            for n in range(8):
                woslab = wpool.tile([P, 8, 512], BF, tag="wqrs", name="woslab")
                wo_src = wo[:, n * 512:(n + 1) * 512].rearrange(
                    "(kc p) m -> p kc m", p=P
                )
                nc.sync.dma_start(out=woslab[:, 0:4, :], in_=wo_src[:, 0:4, :])
                nc.sync.dma_start(out=woslab[:, 4:8, :], in_=wo_src[:, 4:8, :])
                for tt in range(8):
                    ps = cpp.tile([P, 512], F32, tag="cps", name="wo_ps")
                    for kc in range(8):
                        nc.tensor.matmul(
                            ps[:],
                            aout_sb[:, kc, tt * P:(tt + 1) * P],
                            woslab[:, kc, :],
                            start=(kc == 0),
                            stop=(kc == 7),
                        )
                    ot = qkpool.tile([P, 512], F32, tag="qkt", name="eot")
                    nc.any.tensor_copy(out=ot[:], in_=ps[:])
                    nc.sync.dma_start(
                        out=out[tt * P:(tt + 1) * P, n * 512:(n + 1) * 512],
                        in_=ot[:],
                    )

    nc.compile()
    return nc


def _prep_inputs(inputs):
    bf = ml_dtypes.bfloat16
    x = np.asarray(inputs["x"], dtype=np.float32)
    Wq_down = np.asarray(inputs["Wq_down"], dtype=np.float32).astype(bf)
    Wkv_down = np.asarray(inputs["Wkv_down"], dtype=np.float32).astype(bf)
    Wq_up = np.asarray(inputs["Wq_up"], dtype=np.float32).astype(bf)
    Wk_up = np.asarray(inputs["Wk_up"], dtype=np.float32).astype(bf)
    Wv_up = np.asarray(inputs["Wv_up"], dtype=np.float32).astype(bf)
    Wq_rope = np.asarray(inputs["Wq_rope"], dtype=np.float32).astype(bf)
    Wk_rope = np.asarray(inputs["Wk_rope"], dtype=np.float32).astype(bf)
    Wo = np.asarray(inputs["Wo"], dtype=np.float32).astype(bf)

    xT = [np.ascontiguousarray(x[b].T).astype(bf) for b in range(NB)]

    in_maps = []
    for core in range(8):
        b = core // 4
        hg = core % 4
        hs = slice(hg * NHC * DH, (hg + 1) * NHC * DH)        # head-dim cols
        rs = slice(hg * NHC * DR, (hg + 1) * NHC * DR)        # rope cols
        in_maps.append(
            {
                "xT": xT[b],
                "wqd": Wq_down,
                "wkvd": Wkv_down,
                "wqu": np.ascontiguousarray(Wq_up[:, hs]),
                "wku": np.ascontiguousarray(Wk_up[:, hs]),
                "wvu": np.ascontiguousarray(Wv_up[:, hs]),
                "wqr": np.ascontiguousarray(Wq_rope[:, rs]),
                "wkr": Wk_rope,
                "wo": np.ascontiguousarray(Wo[hs, :]),
            }
        )
    return in_maps


def kernel(**inputs):
    if "nc" not in _CACHE:
        _CACHE["nc"] = _build_program()
    nc = _CACHE["nc"]
    in_maps = _prep_inputs(inputs)
    res = run_bass_kernel_spmd(nc, in_maps, list(range(8)))
    out = np.zeros((NB, T, DM), dtype=np.float32)
    for core in range(8):
        out[core // 4] += res.results[core]["out"]
    return out


# revision 7
# speedup vs baseline: 1.1657x; 1.1657x over previous
"""MLA attention kernel for 8 Trainium2 NeuronCores.

Sharding: core i -> batch b = i//4, head group hg = i%4 (32 heads each).
Latent down-projections replicated within a batch group; Wq_up/Wq_rope/
Wk_up/Wv_up/Wo sharded by head.  Host sums the 4 partial outputs per batch.

Device program (identical on all cores, SPMD over different data):
  - all matmuls bf16 with fp32 PSUM accumulation
  - projections computed feature-major (features on partitions) so that
    attention scores S^T[k, q] = kT.T @ qT need no transposes
  - softmax: exp on ScalarE (scale 1/sqrt(96) folded in, no max subtraction:
    scores are ~N(0,1)), denominator via an appended ones-column of V in the
    attn@V matmul, division via DVE fast-reciprocal + gpsimd partition
    broadcast
  - engines execute their queues in order, so the per-group instruction
    stream software-pipelines: group g's attention matmuls (which stall on
    ScalarE exps) are interleaved with group g+1's projection matmuls so
    the PE always has ready work; the Wo epilogue reuses the same tile-pool
    tags so its first weight slab DMA prefetches during the last group
"""

import sys

sys.path.insert(0, "/opt/trn_rl_repo")

import numpy as np
import ml_dtypes

import concourse.bass as bass
import concourse.tile as tile
from concourse import bacc, mybir
from concourse.bass_utils import run_bass_kernel_spmd

P = 128
T = 1024          # tokens per batch
DM = 4096         # d_model
KX = DM // P      # 32 feature chunks of x
LAT = 512         # latent dim
LC = LAT // P     # 4 latent chunks
NHC = 32          # heads per core
DH = 32           # head dim (compressed part)
DR = 64           # rope dim per head
NB = 2            # batch
SCALE = 1.0 / float(np.sqrt(DH + DR))

BF = mybir.dt.bfloat16
F32 = mybir.dt.float32

_CACHE = {}


def _build_program():
    nc = bacc.Bacc("TRN2", target_bir_lowering=False, num_devices=8)

    xT = nc.declare_dram_parameter("xT", [DM, T], BF, isOutput=False)
    wqd = nc.declare_dram_parameter("wqd", [DM, LAT], BF, isOutput=False)
    wkvd = nc.declare_dram_parameter("wkvd", [DM, LAT], BF, isOutput=False)
    wqu = nc.declare_dram_parameter("wqu", [LAT, NHC * DH], BF, isOutput=False)
    wku = nc.declare_dram_parameter("wku", [LAT, NHC * DH], BF, isOutput=False)
    wvu = nc.declare_dram_parameter("wvu", [LAT, NHC * DH], BF, isOutput=False)
    wqr = nc.declare_dram_parameter("wqr", [DM, NHC * DR], BF, isOutput=False)
    wkr = nc.declare_dram_parameter("wkr", [DM, DR], BF, isOutput=False)
    wo = nc.declare_dram_parameter("wo", [NHC * DH, DM], BF, isOutput=False)
    out = nc.declare_dram_parameter("out", [T, DM], F32, isOutput=True)

    from contextlib import ExitStack

    with tile.TileContext(nc) as tc, ExitStack() as octx:
        const = octx.enter_context(tc.tile_pool(name="const", bufs=1))

        # Resident tensors (bf16): x transposed, Wk_rope, projections outputs.
        xT_sb = const.tile([P, KX, T], BF, name="xT_sb")
        xT_r = xT[:].rearrange("(ko p) t -> p ko t", p=P)
        wkr_sb = const.tile([P, KX, DR], BF, name="wkr_sb")
        wvu_sb = const.tile([P, LC, NHC * DH], BF, name="wvu_sb")

        cq_sb = const.tile([P, LC, T], BF, name="cq_sb")      # c_q^T
        ckv_sb = const.tile([P, LC, T], BF, name="ckv_sb")    # c_kv^T
        kr_sb = const.tile([DR, T], BF, name="kr_sb")         # k_rope^T (shared)
        # v token-major, per (key-chunk, head): cols 0:32 = v, col 32 = ones
        v_sb = const.tile([P, 8, NHC, 34], BF, name="v_sb")
        # attention output, feature-major: head h -> [32*(h%4):.., h//4, :]
        aout_sb = const.tile([P, 8, T], BF, name="aout_sb")

        nc.vector.memset(v_sb[:, :, :, 32:33], 1.0)

        with ExitStack() as ctx:
            wpool = ctx.enter_context(tc.tile_pool(name="wpool", bufs=2))
            cpp = ctx.enter_context(tc.tile_pool(name="cpp", bufs=2, space="PSUM"))
            qkpool = ctx.enter_context(tc.tile_pool(name="qkpool", bufs=16))
            ppool = ctx.enter_context(tc.tile_pool(name="ppool", bufs=2))
            spp = ctx.enter_context(tc.tile_pool(name="spp", bufs=2, space="PSUM"))
            avp = ctx.enter_context(tc.tile_pool(name="avp", bufs=2, space="PSUM"))
            rpool = ctx.enter_context(tc.tile_pool(name="rpool", bufs=2))

            # x loaded first (PE starts only once data is in — starting it
            # earlier just runs sparse work at the cold HAM half-clock).
            for kq in range(8):
                nc.sync.dma_start(
                    out=xT_sb[:, kq * 4:(kq + 1) * 4, :],
                    in_=xT_r[:, kq * 4:(kq + 1) * 4, :],
                )
            nc.sync.dma_start(
                out=wkr_sb[:], in_=wkr[:].rearrange("(ko p) d -> p ko d", p=P)
            )
            nc.sync.dma_start(
                out=wvu_sb[:], in_=wvu[:].rearrange("(c p) m -> p c m", p=P)
            )

            # ---- Phase B: latent down-projections (feature-major outputs) ----
            for wd, cdst in ((wqd, cq_sb), (wkvd, ckv_sb)):
                for m in range(LC):
                    wslab = wpool.tile([P, KX, P], BF, tag="wqrs", name="bslab")
                    b_src = wd[:, m * P:(m + 1) * P].rearrange(
                        "(ko p) m -> p ko m", p=P
                    )
                    nc.sync.dma_start(out=wslab[:, 0:16, :], in_=b_src[:, 0:16, :])
                    nc.sync.dma_start(out=wslab[:, 16:KX, :], in_=b_src[:, 16:KX, :])
                    for hf in range(2):
                        ps = cpp.tile([P, 512], F32, tag="cps")
                        for k in range(KX):
                            nc.tensor.matmul(
                                ps[:],
                                wslab[:, k, :],
                                xT_sb[:, k, hf * 512:(hf + 1) * 512],
                                start=(k == 0),
                                stop=(k == KX - 1),
                            )
                        nc.vector.tensor_copy(
                            out=cdst[:, m, hf * 512:(hf + 1) * 512], in_=ps[:]
                        )

            # k_rope^T [64, T]
            for hf in range(2):
                ps = cpp.tile([P, 512], F32, tag="cps")
                for k in range(KX):
                    nc.tensor.matmul(
                        ps[:DR, :],
                        wkr_sb[:, k, :],
                        xT_sb[:, k, hf * 512:(hf + 1) * 512],
                        start=(k == 0),
                        stop=(k == KX - 1),
                    )
                nc.vector.tensor_copy(
                    out=kr_sb[:, hf * 512:(hf + 1) * 512], in_=ps[:DR, :]
                )

            # ---- Phase V: v = c_kv @ Wv_up (token-major), interleaved heads ----
            for tt in range(8):
                for hf in range(2):
                    ps = cpp.tile([P, 512], F32, tag="cps")
                    for lc in range(LC):
                        nc.tensor.matmul(
                            ps[:],
                            ckv_sb[:, lc, tt * P:(tt + 1) * P],
                            wvu_sb[:, lc, hf * 512:(hf + 1) * 512],
                            start=(lc == 0),
                            stop=(lc == LC - 1),
                        )
                    # scatter 16 heads x 32 dims into v_sb[:, tt, h, 0:32]
                    nc.vector.tensor_copy(
                        out=v_sb[:, tt, hf * 16:(hf + 1) * 16, 0:32],
                        in_=ps[:].rearrange("p (h d) -> p h d", h=16),
                    )

            # ---- Phase C+D: project q/k per group of 4 heads, then attend.
            # proj(g) is materialized as a list of closures, each emitting a
            # small burst of PE work; attn(g-1) paces through the list so the
            # PE stream always has independent matmuls between exp-gated
            # score/av matmuls.
            def make_proj(g):
                qt = []
                kt = []
                for j in range(4):
                    qtj = qkpool.tile([P, T], BF, tag="qkt", name=f"qt{g}_{j}")
                    ktj = qkpool.tile([P, T], BF, tag="qkt", name=f"kt{g}_{j}")
                    qt.append(qtj)
                    kt.append(ktj)

                chunks = []

                def kr_copies():
                    for j in range(4):
                        nc.vector.tensor_copy(out=kt[j][0:DR, :], in_=kr_sb[:])
                chunks.append(kr_copies)

                # q_rope: wqr m-slabs 2g, 2g+1 -> heads (2*ms, 2*ms+1) rows 0:64
                for s in range(2):
                    wslab = wpool.tile([P, KX, P], BF, tag="wqrs", name="qrslab")
                    qr_src = wqr[:, (2 * g + s) * P:(2 * g + s + 1) * P].rearrange(
                        "(ko p) m -> p ko m", p=P
                    )

                    def qr_dma(wslab=wslab, qr_src=qr_src):
                        nc.sync.dma_start(out=wslab[:, 0:16, :], in_=qr_src[:, 0:16, :])
                        nc.sync.dma_start(out=wslab[:, 16:KX, :], in_=qr_src[:, 16:KX, :])
                    chunks.append(qr_dma)

                    for hf in range(2):
                        ps = cpp.tile([P, 512], F32, tag="cps", name="qr_ps")
                        for k0 in range(0, KX, 4):
                            def qr_mms(ps=ps, wslab=wslab, hf=hf, k0=k0):
                                for k in range(k0, k0 + 4):
                                    nc.tensor.matmul(
                                        ps[:],
                                        wslab[:, k, :],
                                        xT_sb[:, k, hf * 512:(hf + 1) * 512],
                                        start=(k == 0),
                                        stop=(k == KX - 1),
                                    )
                            chunks.append(qr_mms)

                        def qr_copy(ps=ps, hf=hf, s=s):
                            sl = slice(hf * 512, (hf + 1) * 512)
                            nc.vector.tensor_copy(out=qt[2 * s][0:DR, sl], in_=ps[0:DR, :])
                            nc.vector.tensor_copy(out=qt[2 * s + 1][0:DR, sl], in_=ps[DR:P, :])
                        chunks.append(qr_copy)

                # q_c / k_c: up-projection slab g (128 cols = 4 heads) rows 64:96
                for wu, dst, src in ((wqu, qt, cq_sb), (wku, kt, ckv_sb)):
                    wslab = wpool.tile([P, LC, P], BF, tag="wups", bufs=3, name="upslab")

                    def up_dma(wslab=wslab, wu=wu):
                        nc.sync.dma_start(
                            out=wslab[:],
                            in_=wu[:, g * P:(g + 1) * P].rearrange(
                                "(c p) m -> p c m", p=P
                            ),
                        )
                    chunks.append(up_dma)

                    for hf in range(2):
                        def up_mms_copy(wslab=wslab, dst=dst, src=src, hf=hf):
                            ps = cpp.tile([P, 512], F32, tag="cps", name="up_ps")
                            for lc in range(LC):
                                nc.tensor.matmul(
                                    ps[:],
                                    wslab[:, lc, :],
                                    src[:, lc, hf * 512:(hf + 1) * 512],
                                    start=(lc == 0),
                                    stop=(lc == LC - 1),
                                )
                            sl = slice(hf * 512, (hf + 1) * 512)
                            for j in range(4):
                                nc.vector.tensor_copy(
                                    out=dst[j][DR:DR + DH, sl],
                                    in_=ps[j * DH:(j + 1) * DH, :],
                                )
                        chunks.append(up_mms_copy)

                return chunks, qt, kt

            def emit_attn(g, qt, kt, filler):
                # filler: list of closures to pace through this group's attn
                done = [0]
                points = 8 * 17  # weave points: per unit 8 score + 8 av + 1
                pt = [0]

                def weave():
                    pt[0] += 1
                    want = (pt[0] * len(filler)) // points
                    while done[0] < want:
                        filler[done[0]]()
                        done[0] += 1

                for j in range(4):
                    h = 4 * g + j
                    for qc in range(2):
                        qsl = slice(qc * 512, (qc + 1) * 512)
                        probs = ppool.tile([P, 8, 512], BF, tag="probs", name="probs")
                        for kp in range(4):
                            # two key chunks share a 2-bank psum tile so one
                            # N=1024 exp amortizes ScalarE's ~352-cycle
                            # per-instruction overhead
                            sp = spp.tile([P, 2, 512], F32, tag="sps", name="sps")
                            for ki in range(2):
                                nc.tensor.matmul(
                                    sp[:, ki, :],
                                    kt[j][0:96, (2 * kp + ki) * P:(2 * kp + ki + 1) * P],
                                    qt[j][0:96, qsl],
                                    start=True,
                                    stop=True,
                                )
                                weave()
                            nc.scalar.activation(
                                out=probs[:, 2 * kp:2 * kp + 2, :],
                                in_=sp[:],
                                func=mybir.ActivationFunctionType.Exp,
                                scale=SCALE,
                            )
                        av = avp.tile([33, 512], F32, tag="avp", name="av")
                        for kc in range(8):
                            nc.tensor.matmul(
                                av[:],
                                v_sb[:, kc, h, 0:33],
                                probs[:, kc, :],
                                start=(kc == 0),
                                stop=(kc == 7),
                            )
                            weave()
                        # denominator must be copied out of PSUM before the
                        # bit-trick reciprocal (it needs IEEE fp32 bit layout;
                        # PSUM's e10m23 accumulator bits break BITWISE_NOT)
                        den = rpool.tile([1, 512], F32, tag="dc", name="den")
                        nc.vector.tensor_copy(out=den[:], in_=av[32:33, :])
                        recip = rpool.tile([1, 512], F32, tag="rc", name="recip")
                        nc.vector.reciprocal_approx_fast(out=recip[:], in_=den[:])
                        rrep = rpool.tile([DH, 512], F32, tag="rr", name="rrep")
                        nc.gpsimd.partition_broadcast(rrep[:], recip[:])
                        nc.vector.tensor_mul(
                            out=aout_sb[j * DH:(j + 1) * DH, g, qsl],
                            in0=av[0:DH, :],
                            in1=rrep[:],
                        )
                        weave()
                # drain any leftover filler
                while done[0] < len(filler):
                    filler[done[0]]()
                    done[0] += 1

            chunks, qt, kt = make_proj(0)
            for c in chunks:
                c()
            for g in range(8):
                if g < 7:
                    nchunks, nqt, nkt = make_proj(g + 1)
                else:
                    # last group: prefetch the first Wo slab as filler
                    nchunks = []
                emit_attn(g, qt, kt, nchunks)
                if g < 7:
                    qt, kt = nqt, nkt

            # ---- Phase E: out = aout^T @ Wo (token-major), Wo streamed once.
            # Reuses the wqrs slab slots (8 KiB) and cps psum slots so the
            # first weight DMA prefetches while group 7's attention drains.
            for n in range(8):
                woslab = wpool.tile([P, 8, 512], BF, tag="wqrs", name="woslab")
                wo_src = wo[:, n * 512:(n + 1) * 512].rearrange(
                    "(kc p) m -> p kc m", p=P
                )
                nc.sync.dma_start(out=woslab[:, 0:4, :], in_=wo_src[:, 0:4, :])
                nc.sync.dma_start(out=woslab[:, 4:8, :], in_=wo_src[:, 4:8, :])
                for tt in range(8):
                    ps = cpp.tile([P, 512], F32, tag="cps", name="wo_ps")
                    for kc in range(8):
                        nc.tensor.matmul(
                            ps[:],
                            aout_sb[:, kc, tt * P:(tt + 1) * P],
                            woslab[:, kc, :],
                            start=(kc == 0),
                            stop=(kc == 7),
                        )
                    ot = qkpool.tile([P, 512], F32, tag="qkt", name="eot")
                    nc.any.tensor_copy(out=ot[:], in_=ps[:])
                    nc.sync.dma_start(
                        out=out[tt * P:(tt + 1) * P, n * 512:(n + 1) * 512],
                        in_=ot[:],
                    )

    nc.compile()
    return nc


def _prep_inputs(inputs):
    bf = ml_dtypes.bfloat16
    x = np.asarray(inputs["x"], dtype=np.float32)
    Wq_down = np.asarray(inputs["Wq_down"], dtype=np.float32).astype(bf)
    Wkv_down = np.asarray(inputs["Wkv_down"], dtype=np.float32).astype(bf)
    Wq_up = np.asarray(inputs["Wq_up"], dtype=np.float32).astype(bf)
    Wk_up = np.asarray(inputs["Wk_up"], dtype=np.float32).astype(bf)
    Wv_up = np.asarray(inputs["Wv_up"], dtype=np.float32).astype(bf)
    Wq_rope = np.asarray(inputs["Wq_rope"], dtype=np.float32).astype(bf)
    Wk_rope = np.asarray(inputs["Wk_rope"], dtype=np.float32).astype(bf)
    Wo = np.asarray(inputs["Wo"], dtype=np.float32).astype(bf)

    xT = [np.ascontiguousarray(x[b].T).astype(bf) for b in range(NB)]

    in_maps = []
    for core in range(8):
        b = core // 4
        hg = core % 4
        hs = slice(hg * NHC * DH, (hg + 1) * NHC * DH)        # head-dim cols
        rs = slice(hg * NHC * DR, (hg + 1) * NHC * DR)        # rope cols
        in_maps.append(
            {
                "xT": xT[b],
                "wqd": Wq_down,
                "wkvd": Wkv_down,
                "wqu": np.ascontiguousarray(Wq_up[:, hs]),
                "wku": np.ascontiguousarray(Wk_up[:, hs]),
                "wvu": np.ascontiguousarray(Wv_up[:, hs]),
                "wqr": np.ascontiguousarray(Wq_rope[:, rs]),
                "wkr": Wk_rope,
                "wo": np.ascontiguousarray(Wo[hs, :]),
            }
        )
    return in_maps


def kernel(**inputs):
    if "nc" not in _CACHE:
        _CACHE["nc"] = _build_program()
    nc = _CACHE["nc"]
    in_maps = _prep_inputs(inputs)
    res = run_bass_kernel_spmd(nc, in_maps, list(range(8)))
    out = np.zeros((NB, T, DM), dtype=np.float32)
    for core in range(8):
        out[core // 4] += res.results[core]["out"]
    return out
